# revision 20
# baseline (speedup 1.0000x reference)
"""TRN2 Bass kernel for nn_Attention_78348793414287 (linear attention).

Reference computation (N=4, T=4096, H=16, DM=DA=1024, dh=64; masks all-ones):
  qh = split_heads(q @ Wq); kh = split_heads(k @ Wk); vh = split_heads(v @ Wv)
  k_sm = softmax(kh, axis=t);  kv = einsum('nhtd,nhte->nhde', k_sm, vh)
  q_sm = softmax(qh, axis=d);  out = einsum('nhtd,nhde->nhte', q_sm, kv)

Sharding: 8 cores = 4 batches x 2 head-groups (8 heads / 512 cols per core).
Per-core layout: q/k/v are fed host-transposed as [DM, T] so the tensor engine
consumes them directly; all matmuls run as fp32r (TF32, full rate) with fp32
PSUM accumulation. The q-softmax (over d) is done in transposed layout via a
selector matmul (64-row group sums) + K=2 broadcast matmul; the k-softmax
(over t) folds into the kv matmul via an appended ones-column (column sums)
and a per-partition reciprocal scale of kv.
"""
import sys

import numpy as np

sys.path.insert(0, "/opt/trn_rl_repo")

import concourse.bacc as bacc
import concourse.mybir as mybir
from concourse import tile
from concourse.bass_utils import run_bass_kernel_spmd

F32 = mybir.dt.float32
F32R = mybir.dt.float32r
BF16 = mybir.dt.bfloat16
FP16 = mybir.dt.float16
AFT = mybir.ActivationFunctionType
ALU = mybir.AluOpType

N, T, H, DM = 4, 4096, 16, 1024
C = 512          # columns (= 8 heads x 64) per core
NCORES = 8
TCH = T // 512   # 8 t-chunks of 512
DMC = DM // 128  # 8 contraction chunks
NCT = C // 128   # 4 col-tiles (head pairs)


def _round_tf32(x: np.ndarray) -> np.ndarray:
    """Round fp32 to tf32 (10-bit mantissa), round-to-nearest-even."""
    b = np.ascontiguousarray(x, dtype=np.float32).view(np.uint32)
    lsb = (b >> np.uint32(13)) & np.uint32(1)
    b = (b + np.uint32(0x0FFF) + lsb) & np.uint32(0xFFFFE000)
    return b.view(np.float32)


def _patch_act_tables():
    """Steer both Exp and Ln onto the shared natural_log_exp_and_others ACT
    table (same 400-bucket precision) so the scheduler emits one table load
    instead of reloading on every Exp<->Ln switch (~1.3us each)."""
    if getattr(bacc, "_act_tables_patched", False):
        return
    orig = bacc.get_activation_tables

    def patched(arch):
        tables = dict(orig(arch))
        exp_t = mybir.ActivationFunctionType.Exp
        ln_t = mybir.ActivationFunctionType.Ln
        if "natural_log_exp_and_others" in tables:
            for name, funcs in tables.items():
                if name != "natural_log_exp_and_others":
                    tables[name] = funcs - {exp_t, ln_t}
        return tables

    bacc.get_activation_tables = patched
    bacc._act_tables_patched = True


def _build():
    _patch_act_tables()
    nc = bacc.Bacc("TRN2", target_bir_lowering=False, debug=False)
    qT_d = nc.dram_tensor("qT", [DM, T], FP16, kind="ExternalInput").ap()
    kT_d = nc.dram_tensor("kT", [DM, T], FP16, kind="ExternalInput").ap()
    vT_d = nc.dram_tensor("vT", [DM, T], FP16, kind="ExternalInput").ap()
    wq_d = nc.dram_tensor("wq", [DM, C], FP16, kind="ExternalInput").ap()
    wk_d = nc.dram_tensor("wk", [DM, C], FP16, kind="ExternalInput").ap()
    wv_d = nc.dram_tensor("wv", [DM, C], FP16, kind="ExternalInput").ap()
    sel_sum_d = nc.dram_tensor("sel_sum", [128, 2], FP16, kind="ExternalInput").ap()
    sel_bc_d = nc.dram_tensor("sel_bc", [2, 128], FP16, kind="ExternalInput").ap()
    neg4_d = nc.dram_tensor("neg4", [128, 1], F32, kind="ExternalInput").ap()
    outT_d = nc.dram_tensor("outT", [C, T], F32, kind="ExternalOutput").ap()

    with tile.TileContext(nc) as tc:
        with (
            tc.tile_pool(name="weights", bufs=1) as wpool,
            tc.tile_pool(name="stream", bufs=4) as stream,
            tc.tile_pool(name="acts", bufs=2) as acts,
            tc.tile_pool(name="small", bufs=1) as small,
            tc.tile_pool(name="pswork", bufs=2, space="PSUM") as pswork,
            tc.tile_pool(name="pskv", bufs=1, space="PSUM") as pskv,
        ):
            wk_sb = wpool.tile([128, DMC, C], FP16, tag="wk")
            wv_sb = wpool.tile([128, DMC, C], FP16, tag="wv")
            wq_sb = wpool.tile([128, DMC, C], FP16, tag="wq")
            wk_r = wk_d.rearrange("(c p) n -> p c n", p=128)
            wv_r = wv_d.rearrange("(c p) n -> p c n", p=128)
            for dm in range(DMC):
                nc.scalar.dma_start(wk_sb[:, dm, :], wk_r[:, dm, :])
            for dm in range(DMC):
                nc.gpsimd.dma_start(wv_sb[:, dm, :], wv_r[:, dm, :])
            sel_sum = small.tile([128, 2], FP16, tag="sel_sum")
            sel_bc = small.tile([2, 128], FP16, tag="sel_bc")
            nc.gpsimd.dma_start(sel_sum[:], sel_sum_d[:])
            nc.gpsimd.dma_start(sel_bc[:], sel_bc_d[:])
            neg4 = small.tile([128, 1], F32, tag="neg4")
            nc.gpsimd.dma_start(neg4[:], neg4_d[:])
            nc.gpsimd.dma_start(wq_sb[:], wq_d.rearrange("(c p) n -> p c n", p=128))

            # kv block-diagonal stationary tiles for the final einsum
            kv_sb = [
                small.tile([128, 128], FP16, tag=f"kv{p}", name=f"kv{p}")
                for p in range(NCT)
            ]

            # ---- Phase A: kh/vh projections, exp(kh), kv + column sums ----
            if True:
                kvbank = [
                    pskv.tile([128, 260], F32, name=f"kvbank{b}") for b in range(2)
                ]
                kvps = [kvbank[p // 2][:, (p % 2) * 130 : (p % 2) * 130 + 130]
                        for p in range(NCT)]
                for ch in range(TCH):
                    ksb = stream.tile([128, DMC, 512], FP16, tag="k")
                    vsb = stream.tile([128, DMC, 512], FP16, tag="v")
                    tsl = slice(ch * 512, (ch + 1) * 512)
                    nc.sync.dma_start(
                        ksb[:], kT_d.rearrange("(c p) t -> p c t", p=128)[:, :, tsl]
                    )
                    nc.sync.dma_start(
                        vsb[:], vT_d.rearrange("(c p) t -> p c t", p=128)[:, :, tsl]
                    )
                    for tt in range(4):
                        ts128 = slice(tt * 128, (tt + 1) * 128)
                        kh_ps = pswork.tile([128, 512], F32, tag="work")
                        for dm in range(DMC):
                            nc.tensor.matmul(
                                kh_ps[:],
                                ksb[:, dm, ts128],
                                wk_sb[:, dm, :],
                                start=(dm == 0),
                                stop=(dm == DMC - 1),
                            )
                        ek = acts.tile([128, 512], FP16, tag="ek")
                        nc.scalar.activation(ek[:], kh_ps[:], AFT.Exp)

                        vh_ps = pswork.tile([128, 512], F32, tag="work")
                        for dm in range(DMC):
                            nc.tensor.matmul(
                                vh_ps[:],
                                vsb[:, dm, ts128],
                                wv_sb[:, dm, :],
                                start=(dm == 0),
                                stop=(dm == DMC - 1),
                            )
                        # vh_aug[p, pair, 0:128] = vh block; cols 128:130 = 1.0
                        vh_aug = acts.tile([128, NCT, 130], FP16, tag="vh")
                        nc.vector.tensor_copy(
                            vh_aug[:, :, 0:128],
                            vh_ps[:].rearrange("p (c n) -> p c n", c=NCT),
                        )
                        # ones columns: 0*x + 1 (memset can't produce f32r)
                        nc.vector.tensor_scalar(
                            vh_aug[:, :, 128:130],
                            vh_ps[:, 0:8].rearrange("p (c n) -> p c n", c=NCT),
                            0.0,
                            1.0,
                            op0=ALU.mult,
                            op1=ALU.add,
                        )

                        first = ch == 0 and tt == 0
                        last = ch == TCH - 1 and tt == 3
                        for p in range(NCT):
                            # start=True clears has_written for the whole PSUM
                            # bank; only the bank's first matmul may set it.
                            # The odd pair's first write lands on cleared
                            # has_written bits and overwrites by itself.
                            nc.tensor.matmul(
                                kvps[p][:],
                                ek[:, p * 128 : (p + 1) * 128],
                                vh_aug[:, p, :],
                                start=first and p % 2 == 0,
                                stop=last and p % 2 == 1,
                                skip_group_check=True,
                            )

                # kv rows scaled by 1/S_k (col 128 holds S_k), block-diagonal
                for p in range(NCT):
                    rk = small.tile([128, 1], F32, tag=f"rk{p}", name=f"rk{p}")
                    with nc.allow_low_precision(reason="softmax reciprocal"):
                        nc.vector.reciprocal(rk[:], kvps[p][:, 128:129])
                    for half in range(2):
                        h64 = slice(half * 64, (half + 1) * 64)
                        o64 = slice((1 - half) * 64, (2 - half) * 64)
                        nc.vector.tensor_scalar(
                            kv_sb[p][h64, h64],
                            kvps[p][h64, h64],
                            rk[h64, :],
                            0.03125,
                            op0=ALU.mult,
                            op1=ALU.mult,
                        )
                        # off-diagonal cross-head block: zero via 0*x
                        nc.vector.tensor_scalar(
                            kv_sb[p][h64, o64],
                            kvps[p][h64, o64],
                            0.0,
                            None,
                            op0=ALU.mult,
                        )

            # ---- Phase B: qh projection, q-softmax via exp(qh - ln S), out ----
            with (
                tc.tile_pool(name="psqh", bufs=3, space="PSUM") as psqh,
                tc.tile_pool(name="pssm", bufs=1, space="PSUM") as pssm,
            ):
                for ch in range(TCH):
                    qsb = stream.tile([128, DMC, 512], FP16, tag="q")
                    tsl = slice(ch * 512, (ch + 1) * 512)
                    nc.sync.dma_start(
                        qsb[:], qT_d.rearrange("(c p) t -> p c t", p=128)[:, :, tsl]
                    )
                    for ct in range(NCT):
                        qh_ps = psqh.tile([128, 512], F32, tag="qh")
                        for dm in range(DMC):
                            nc.tensor.matmul(
                                qh_ps[:],
                                wq_sb[:, dm, ct * 128 : (ct + 1) * 128],
                                qsb[:, dm, :],
                                start=(dm == 0),
                                stop=(dm == DMC - 1),
                            )
                        eq = acts.tile([128, 512], FP16, tag="eq")
                        nc.scalar.activation(eq[:], qh_ps[:], AFT.Exp)

                        sq_ps = pssm.tile([2, 512], F32, tag="sm")
                        nc.tensor.matmul(
                            sq_ps[:], sel_sum[:], eq[:], start=True, stop=True
                        )
                        # lq = ln(S * e^-4) = ln(S) - 4: centered so tf32
                        # rounding of lq costs ~3e-4 instead of ~2e-3
                        lq = acts.tile([2, 512], FP16, tag="lq")
                        nc.scalar.activation(lq[:], sq_ps[:], AFT.Ln,
                                             scale=0.018315638888734179)
                        # qh_ps -= broadcast(ln S): sel_bc holds -1 entries
                        nc.tensor.matmul(
                            qh_ps[:], sel_bc[:], lq[:],
                            start=False, stop=True, skip_group_check=True,
                        )
                        eq2 = acts.tile([128, 512], FP16, tag="eq2")
                        nc.scalar.activation(eq2[:], qh_ps[:], AFT.Exp, bias=neg4[:])

                        o_ps = pswork.tile([128, 512], F32, tag="work")
                        nc.tensor.matmul(
                            o_ps[:], kv_sb[ct][:], eq2[:], start=True, stop=True
                        )
                        osb = acts.tile([128, 512], F32, tag="osb")
                        nc.vector.tensor_copy(osb[:], o_ps[:])
                        nc.scalar.dma_start(
                            outT_d[ct * 128 : (ct + 1) * 128, tsl], osb[:]
                        )

    nc.compile()
    return nc


_NC_CACHE = None


def _get_nc():
    global _NC_CACHE
    if _NC_CACHE is None:
        _NC_CACHE = _build()
    return _NC_CACHE


def _make_in_maps(q, k, v, Wq, Wk, Wv):
    sel_sum = np.zeros((128, 2), np.float16)
    sel_sum[0:64, 0] = 1.0
    sel_sum[64:128, 1] = 1.0
    # negated: used to subtract the broadcast ln(S) inside the qh PSUM
    sel_bc = np.zeros((2, 128), np.float16)
    sel_bc[0, 0:64] = -1.0
    sel_bc[1, 64:128] = -1.0
    neg4 = np.full((128, 1), -4.0 + np.log(32.0), np.float32)

    f16 = np.float16
    wq_r = [np.ascontiguousarray(Wq[:, g * C : (g + 1) * C]).astype(f16) for g in range(2)]
    wk_r = [np.ascontiguousarray(Wk[:, g * C : (g + 1) * C]).astype(f16) for g in range(2)]
    wv_r = [np.ascontiguousarray(Wv[:, g * C : (g + 1) * C]).astype(f16) for g in range(2)]
    qT = [np.ascontiguousarray(np.asarray(q[n]).T).astype(f16) for n in range(N)]
    kT = [np.ascontiguousarray(np.asarray(k[n]).T).astype(f16) for n in range(N)]
    vT = [np.ascontiguousarray(np.asarray(v[n]).T).astype(f16) for n in range(N)]

    in_maps = []
    for core in range(NCORES):
        n, g = core // 2, core % 2
        in_maps.append(
            {
                "qT": qT[n], "kT": kT[n], "vT": vT[n],
                "wq": wq_r[g], "wk": wk_r[g], "wv": wv_r[g],
                "sel_sum": sel_sum, "sel_bc": sel_bc, "neg4": neg4,
            }
        )
    return in_maps


def run(q, k, v, Wq, Wk, Wv, trace=False, trace_cores=None):
    nc = _get_nc()
    in_maps = _make_in_maps(q, k, v, Wq, Wk, Wv)
    res = run_bass_kernel_spmd(
        nc, in_maps, list(range(NCORES)), trace=trace, trace_cores=trace_cores
    )
    out = np.empty((N, T, H * 64), np.float32)
    for core in range(NCORES):
        n, g = core // 2, core % 2
        out[n, :, g * C : (g + 1) * C] = res.results[core]["outT"].T
    return out, res


def kernel(q, k, v, Wq, Wk, Wv, mask_q=None, mask_attn=None, **_unused):
    out, _ = run(
        np.asarray(q, np.float32), np.asarray(k, np.float32),
        np.asarray(v, np.float32), np.asarray(Wq, np.float32),
        np.asarray(Wk, np.float32), np.asarray(Wv, np.float32),
    )
    return out
